# revision 37
# baseline (speedup 1.0000x reference)
"""GRU + MLP head kernel for Trainium2, data-parallel over batch across 8 NeuronCores.

Math (per core, batch slice BL=32):
  xr/xz/xn projections: xg[t] = W_xg @ x[t]  (PE, weight-stationary, chunked over time)
  recurrence (T=512 steps, hT layout [H->2x128 partitions, B free]):
    psum_rz = Id @ [xr_t|xz_t] ; psum_rz += W_hrz @ h       (PE)
    r = sigmoid(psum_r) ; zb = sigmoid(-psum_z) = 1-z       (ACT)
    rh = r*h                                                 (DVE)
    psum_n = Id @ xn_t ; psum_n += W_hn @ rh                 (PE)
    ht = tanh(psum_n)                                        (ACT)
    h = h + zb*(ht - h)                                      (DVE)
  head: sigmoid(h) -> w1 -> sigmoid -> w2 -> softmax
All matmul inputs bf16, PSUM accumulation fp32.
"""

import numpy as np
import ml_dtypes

BF = ml_dtypes.bfloat16
P = 128
B, T, I, H = 256, 512, 128, 256
NCORES = 8
BL = B // NCORES  # batch per core

_prog_cache = {}


def _fuse_ldweights(nc, mybir):
    """Re-fuse standalone InstLdweights into the following InstMatmult.

    Tile lowering splits every matmul into LDWEIGHTS + MATMUL; walrus only
    applies Fast Weight Load (4 XBUS, ~4x faster weight streaming) to
    self-loading matmuls, so the split leaves LDWEIGHTS at 1 col/cycle
    (~104ns per 128-col block) as the dominant PE cost in this kernel.
    Merging the pair back (ldweights=True + wait/dep transfer) lets codegen
    pick FWL. Runs pre-bacc so generate_event_semaphores re-legalizes waits.
    """
    for f in nc.m.functions:
        for bb in f.blocks:
            insts = bb.instructions
            to_remove = []
            i = 0
            n = len(insts)
            while i < n:
                ins = insts[i]
                if isinstance(ins, mybir.InstLdweights):
                    # partner matmul = next PE instruction in block order
                    j = i + 1
                    mm = None
                    while j < n:
                        nxt = insts[j]
                        if isinstance(nxt, mybir.InstMatmult):
                            mm = nxt
                            break
                        if isinstance(nxt, mybir.InstLdweights):
                            break
                        j += 1
                    if mm is not None:
                        lw = ins.sync_info
                        msi = mm.sync_info
                        waits = (list(lw.on_wait) if lw else []) + (
                            list(msi.on_wait) if msi else [])
                        upds = (list(msi.on_update) if msi else []) + (
                            list(lw.on_update) if lw else [])
                        mm.sync_info = mybir.SyncInfo(on_wait=waits, on_update=upds)
                        mm.ldweights = True
                        mm.merge_dependencies_from(ins)
                        to_remove.append(ins)
                i += 1
            for ins in to_remove:
                insts.remove(ins)


def _dedup_adjacent_ldweights(nc, mybir):
    """Remove InstLdweights that reload the exact weights AP already loaded.

    Runs on the final BIR (post nc.compile()). Walks each block's PE weight
    stream; an InstLdweights whose weights AP string-matches the previous
    load, with only InstMatmult between, is redundant — the following
    matmuls (ldweights=False post-split) use whatever is loaded. Waits on
    the removed load migrate to the next matmul when it has none (each
    instruction may carry at most one wait); pairs where both carry waits
    are left alone. Weight SBUF regions are write-once in this kernel, so
    same-AP implies same contents.
    """
    removed = 0
    for f in nc.m.functions:
        for bb in f.blocks:
            insts = bb.instructions
            last_ap = None
            last_clobber = False
            to_remove = []
            for idx, ins in enumerate(insts):
                if isinstance(ins, mybir.InstMatmult):
                    continue
                if not isinstance(ins, mybir.InstLdweights):
                    continue
                ap_s = str(ins.ins[0])
                if ap_s == last_ap:
                    si = ins.sync_info
                    waits = list(si.on_wait) if si else []
                    upds = list(si.on_update) if si else []
                    # find following matmul to absorb waits/updates
                    mm = None
                    for j in range(idx + 1, len(insts)):
                        if isinstance(insts[j], mybir.InstMatmult):
                            mm = insts[j]
                            break
                        if isinstance(insts[j], mybir.InstLdweights):
                            break
                    if mm is None:
                        last_ap = ap_s
                        continue
                    msi = mm.sync_info
                    mw = list(msi.on_wait) if msi else []
                    mu = list(msi.on_update) if msi else []
                    if waits and mw:
                        last_ap = ap_s
                        continue  # can't merge two waits; keep the load
                    mm.sync_info = mybir.SyncInfo(on_wait=waits + mw,
                                                  on_update=mu + upds)
                    mm.merge_dependencies_from(ins)
                    to_remove.append(ins)
                    removed += 1
                else:
                    last_ap = ap_s
            for ins in to_remove:
                insts.remove(ins)
    return removed


def _build_program(T_total, chunk, body, chains, act_copy_bias):
    import concourse.bacc as bacc
    import concourse.tile as tile
    import concourse.bass as bass
    from concourse import mybir

    f32 = mybir.dt.float32
    bf16 = mybir.dt.bfloat16
    AF = mybir.ActivationFunctionType
    ET = mybir.EngineType

    Bc = BL // chains
    nchunks = T_total // chunk
    nblk = chunk // 16  # projection blocks (16 timesteps each) per chunk

    nc = bacc.Bacc("TRN2", target_bir_lowering=False, debug=False)

    xT_d = nc.dram_tensor("xT", [P, BL * T_total], bf16, kind="ExternalInput")
    wrz_d = nc.dram_tensor("wrz", [P, 8 * P], bf16, kind="ExternalInput")
    wn_d = nc.dram_tensor("wn", [P, 4 * P], bf16, kind="ExternalInput")
    wx_d = nc.dram_tensor("wx", [P, 6 * P], bf16, kind="ExternalInput")
    ident_d = nc.dram_tensor("ident", [P, P], bf16, kind="ExternalInput")
    bproj_d = nc.dram_tensor("bproj", [P, 6], f32, kind="ExternalInput")
    w1T_d = nc.dram_tensor("w1T", [P, 2 * 64], bf16, kind="ExternalInput")
    w2T_d = nc.dram_tensor("w2T", [64, 10], bf16, kind="ExternalInput")
    b1_d = nc.dram_tensor("b1", [64, 1], f32, kind="ExternalInput")
    b2_d = nc.dram_tensor("b2", [1, 10], f32, kind="ExternalInput")
    out_d = nc.dram_tensor("out", [BL, 10], f32, kind="ExternalOutput")

    with tile.TileContext(nc) as tc:
        with (
            tc.tile_pool(name="consts", bufs=1) as consts,
            tc.tile_pool(name="xrzp", bufs=1) as xrzp,
            tc.tile_pool(name="state", bufs=1) as state,
            tc.tile_pool(name="work", bufs=3) as work,
            tc.tile_pool(name="ppsum", bufs=2, space="PSUM") as ppsum,
            tc.tile_pool(name="rzpsum", bufs=2 * chains, space="PSUM") as rzpsum,
            tc.tile_pool(name="npsum", bufs=2, space="PSUM") as npsum,
        ):
            # ---- load constants ----
            # xT is t-major [P, (t b)]; one DMA per chunk so chunk-0
            # projections start after ~1MB instead of the full 4MB.
            xT_sb = consts.tile([P, BL * T_total], bf16)
            for ci in range(nchunks):
                cw = chunk * BL
                nc.sync.dma_start(out=xT_sb[:, ci * cw : (ci + 1) * cw],
                                  in_=xT_d.ap()[:, ci * cw : (ci + 1) * cw])
            wrz_sb = consts.tile([P, 8 * P], bf16)
            nc.sync.dma_start(out=wrz_sb, in_=wrz_d.ap())
            wn_sb = consts.tile([P, 4 * P], bf16)
            nc.sync.dma_start(out=wn_sb, in_=wn_d.ap())
            wx_sb = consts.tile([P, 6 * P], bf16)
            nc.sync.dma_start(out=wx_sb, in_=wx_d.ap())
            id_sb = consts.tile([P, P], bf16)
            nc.sync.dma_start(out=id_sb, in_=ident_d.ap())
            bproj_sb = consts.tile([P, 6], f32)
            nc.sync.dma_start(out=bproj_sb, in_=bproj_d.ap())
            w1T_sb = consts.tile([P, 2 * 64], bf16)
            nc.sync.dma_start(out=w1T_sb, in_=w1T_d.ap())
            w2T_sb = consts.tile([64, 10], bf16)
            nc.sync.dma_start(out=w2T_sb, in_=w2T_d.ap())
            b1_sb = consts.tile([64, 1], f32)
            nc.sync.dma_start(out=b1_sb, in_=b1_d.ap())
            b2_sb = consts.tile([1, 10], f32)
            nc.sync.dma_start(out=b2_sb, in_=b2_d.ap())
            ones_sb = consts.tile([1, BL], f32)
            nc.vector.memset(ones_sb, 1.0)

            # xT viewed [P, t, b]
            xT_v = xT_sb.rearrange("p (t b) -> p t b", b=BL)

            # xrz double buffers: [P, chunk(t), 6(gc), 32(b)] bf16
            xrz_bufs = [xrzp.tile([P, chunk, 6, BL], bf16, tag=f"xrz{i}", name=f"xrz{i}")
                        for i in range(2)]

            # hidden state ping-pong per chain
            h_tiles = [
                [state.tile([P, 2 * Bc], bf16, tag=f"h{c}_{i}", name=f"h{c}_{i}")
                 for i in range(2)]
                for c in range(chains)
            ]
            for c in range(chains):
                nc.vector.memset(h_tiles[c][0], 0.0)

            def emit_proj(cidx):
                """Projections for chunk cidx into xrz_bufs[cidx % 2]."""
                buf = xrz_bufs[cidx % 2]
                t0 = cidx * chunk
                # blk-outer so the earliest timesteps' gates finish first and
                # the recurrence can start while later blocks still project.
                for blk in range(nblk):
                    for gc in range(6):
                        g, mc = gc // 2, gc % 2
                        pp = ppsum.tile([P, 16 * BL], f32, tag="pp")
                        # rhs (t, b)-major (xT layout) -> psum cols t*BL+b:
                        # contiguous stream in, stride-friendly copy out.
                        rhs = xT_v[:, t0 + blk * 16 : t0 + (blk + 1) * 16, :]
                        nc.tensor.matmul(
                            pp, wx_sb[:, gc * P : (gc + 1) * P], rhs,
                            start=True, stop=True,
                        )
                        # copy psum [P, t, b] -> xrz[t, gc, b]; both (t, b) ordered
                        src = pp.rearrange("p (t b) -> p t b", t=16)
                        dst = buf[:, blk * 16 : (blk + 1) * 16, gc, :]
                        if act_copy_bias:
                            nc.scalar.activation(
                                dst, src, AF.Copy, bias=bproj_sb[:, gc : gc + 1]
                            )
                        elif (blk * 6 + gc) % 2 == 0:
                            nc.scalar.activation(dst, src, AF.Copy)
                        else:
                            nc.vector.tensor_copy(dst, src)

            def emit_step(tsl, xbuf, c, h_in, h_out):
                """One GRU step for chain c at dynamic time slice tsl, reading xbuf."""
                b0 = c * Bc
                # rz phase
                prz = rzpsum.tile([P, 4 * Bc], f32, tag="prz")
                rz_rhs = xbuf[:, tsl, 0:4, b0 : b0 + Bc]
                nc.tensor.matmul(prz, id_sb, rz_rhs, start=True, stop=False,
                                 skip_group_check=True)
                for mc in range(4):
                    for k in range(2):
                        nc.tensor.matmul(
                            prz[:, mc * Bc : (mc + 1) * Bc],
                            wrz_sb[:, (k * 4 + mc) * P : (k * 4 + mc + 1) * P],
                            h_in[:, k * Bc : (k + 1) * Bc],
                            start=False, stop=(k == 1),
                            skip_group_check=True,
                        )
                # z-gate weights are pre-negated host-side, so one sigmoid
                # yields [r | 1-z] together.
                rzb_sb = work.tile([P, 4 * Bc], bf16, tag=f"rzb{c}")
                nc.scalar.activation(rzb_sb, prz, AF.Sigmoid)
                r_sb = rzb_sb[:, 0 : 2 * Bc]
                zb_sb = rzb_sb[:, 2 * Bc : 4 * Bc]
                rh_sb = work.tile([P, 2 * Bc], bf16, tag=f"rh{c}")
                nc.vector.tensor_mul(rh_sb, r_sb, h_in)
                # n phase
                pn = npsum.tile([P, 2 * Bc], f32, tag="pn")
                n_rhs = xbuf[:, tsl, 4:6, b0 : b0 + Bc]
                nc.tensor.matmul(pn, id_sb, n_rhs, start=True, stop=False,
                                 skip_group_check=True)
                for mc in range(2):
                    for k in range(2):
                        nc.tensor.matmul(
                            pn[:, mc * Bc : (mc + 1) * Bc],
                            wn_sb[:, (k * 2 + mc) * P : (k * 2 + mc + 1) * P],
                            rh_sb[:, k * Bc : (k + 1) * Bc],
                            start=False, stop=(k == 1),
                            skip_group_check=True,
                        )
                ht_sb = work.tile([P, 2 * Bc], bf16, tag=f"ht{c}")
                nc.scalar.activation(ht_sb, pn, AF.Tanh)
                u_sb = work.tile([P, 2 * Bc], bf16, tag=f"u{c}")
                nc.vector.tensor_sub(u_sb, ht_sb, h_in)
                d_sb = work.tile([P, 2 * Bc], bf16, tag=f"d{c}")
                nc.vector.tensor_mul(d_sb, zb_sb, u_sb)
                nc.vector.tensor_add(h_out, h_in, d_sb)

            # ---- main schedule ----
            emit_proj(0)
            import concourse.bass as _b

            for cidx in range(nchunks):
                if cidx + 1 < nchunks:
                    emit_proj(cidx + 1)
                xbuf = xrz_bufs[cidx % 2]
                if body == 0:
                    # fully static unroll: cheap APs, no back-edge barriers
                    for s in range(chunk):
                        tsl = slice(s, s + 1)
                        for c in range(chains):
                            h_in = h_tiles[c][s % 2]
                            h_out = h_tiles[c][(s + 1) % 2]
                            emit_step(tsl, xbuf, c, h_in, h_out)
                else:
                    with tc.For_i(0, chunk, body, hint_engines=(ET.PE,)) as tv:
                        for s in range(body):
                            t_sv = tv + s if s > 0 else tv
                            tsl = _b.ds(t_sv, 1)
                            for c in range(chains):
                                h_in = h_tiles[c][s % 2]
                                h_out = h_tiles[c][(s + 1) % 2]
                                emit_step(tsl, xbuf, c, h_in, h_out)

            # ---- head ----
            for c in range(chains):
                h_fin = h_tiles[c][0]
                sh = work.tile([P, 2 * Bc], bf16, tag=f"sh{c}")
                nc.scalar.activation(sh, h_fin, AF.Sigmoid)
                p1 = npsum.tile([64, Bc], f32, tag="pn")
                for k in range(2):
                    nc.tensor.matmul(
                        p1, w1T_sb[:, k * 64 : (k + 1) * 64],
                        sh[:, k * Bc : (k + 1) * Bc],
                        start=(k == 0), stop=(k == 1), skip_group_check=True,
                    )
                s1 = work.tile([64, Bc], bf16, tag=f"s1{c}")
                nc.scalar.activation(s1, p1, AF.Sigmoid, bias=b1_sb)
                p2 = npsum.tile([Bc, 10], f32, tag="pn")
                nc.tensor.matmul(p2, ones_sb[:, c * Bc : (c + 1) * Bc], b2_sb,
                                 start=True, stop=False, skip_group_check=True)
                nc.tensor.matmul(p2, s1, w2T_sb, start=False, stop=True,
                                 skip_group_check=True)
                # softmax over free dim (10)
                lg = work.tile([Bc, 10], f32, tag=f"lg{c}")
                nc.vector.tensor_copy(lg, p2)
                mx = work.tile([Bc, 1], f32, tag=f"mx{c}")
                nc.vector.reduce_max(mx, lg, axis=mybir.AxisListType.X)
                nmx = work.tile([Bc, 1], f32, tag=f"nmx{c}")
                nc.vector.tensor_scalar_mul(nmx, mx, -1.0)
                ex = work.tile([Bc, 10], f32, tag=f"ex{c}")
                nc.scalar.activation(ex, lg, AF.Exp, bias=nmx)
                sm = work.tile([Bc, 1], f32, tag=f"sm{c}")
                nc.vector.reduce_sum(sm, ex, axis=mybir.AxisListType.X)
                ri = work.tile([Bc, 1], f32, tag=f"ri{c}")
                nc.vector.reciprocal(ri, sm)
                oo = work.tile([Bc, 10], f32, tag=f"oo{c}")
                nc.vector.tensor_scalar_mul(oo, ex, ri)
                nc.sync.dma_start(out=out_d.ap()[c * Bc : (c + 1) * Bc, :], in_=oo)

    nc.compile()
    _dedup_adjacent_ldweights(nc, mybir)
    return nc


def _get_program(T_total=T, chunk=128, body=0, chains=2, act_copy_bias=False):
    key = (T_total, chunk, body, chains, act_copy_bias)
    if key not in _prog_cache:
        _prog_cache[key] = _build_program(T_total, chunk, body, chains, act_copy_bias)
    return _prog_cache[key]


def _pack_inputs(x, weight_xr, weight_hr, bias_r, weight_xz, weight_hz, bias_z,
                 weight_x, weight_h, bias, w1, b1, w2, b2, T_total=T):
    """Host-side prep: per-core input dicts."""
    def wt_blocks(W, korder):
        # lhsT blocks [P, nk*nm... ] for W [M, K]: block (k, mc) = W[mc*128:(mc+1)*128, k*128:(k+1)*128].T
        blocks = []
        for k, mc in korder:
            blocks.append(np.ascontiguousarray(
                W[mc * P : (mc + 1) * P, k * P : (k + 1) * P].T.astype(BF)))
        return np.concatenate(blocks, axis=1)

    # wrz: order (k*4+mc): mc 0,1 -> hr rows 0/1; mc 2,3 -> -hz rows 0/1
    # (z-gate negated so sigmoid(psum_z) = 1-z directly)
    wrz_blocks = []
    for k in range(2):
        for mc in range(4):
            W = weight_hr if mc < 2 else -weight_hz
            m = mc % 2
            wrz_blocks.append(np.ascontiguousarray(
                W[m * P : (m + 1) * P, k * P : (k + 1) * P].T.astype(BF)))
    wrz = np.concatenate(wrz_blocks, axis=1)
    wn_blocks = []
    for k in range(2):
        for mc in range(2):
            wn_blocks.append(np.ascontiguousarray(
                weight_h[mc * P : (mc + 1) * P, k * P : (k + 1) * P].T.astype(BF)))
    wn = np.concatenate(wn_blocks, axis=1)
    wx_blocks = []
    for g, W in enumerate([weight_xr, -weight_xz, weight_x]):
        for mc in range(2):
            wx_blocks.append(np.ascontiguousarray(
                W[mc * P : (mc + 1) * P, :].T.astype(BF)))
    wx = np.concatenate(wx_blocks, axis=1)

    ident = np.eye(P, dtype=BF)
    bproj = np.stack([bias_r[:P], bias_r[P:], -bias_z[:P], -bias_z[P:],
                      bias[:P], bias[P:]], axis=1).astype(np.float32)
    w1T = np.concatenate([np.ascontiguousarray(w1[:, k * P : (k + 1) * P].T.astype(BF))
                          for k in range(2)], axis=1)  # [128, 2*64]
    w2T = np.ascontiguousarray(w2.T.astype(BF))  # [64, 10]
    b1c = b1.reshape(64, 1).astype(np.float32)
    b2c = b2.reshape(1, 10).astype(np.float32)

    common = dict(wrz=wrz, wn=wn, wx=wx, ident=ident, bproj=bproj,
                  w1T=w1T, w2T=w2T, b1=b1c, b2=b2c)
    in_maps = []
    for c in range(NCORES):
        xs = x[c * BL : (c + 1) * BL, :T_total, :].astype(BF)  # [BL, T, I]
        # t-major: [I, T, BL] -> [P, T*BL]
        xTc = np.ascontiguousarray(xs.transpose(2, 1, 0).reshape(P, BL * T_total))
        in_maps.append(dict(common, xT=xTc))
    return in_maps


def kernel(x, weight_xr, weight_hr, bias_r, weight_xz, weight_hz, bias_z,
           weight_x, weight_h, bias, w1, b1, w2, b2):
    from concourse.bass_utils import run_bass_kernel_spmd

    x = np.asarray(x, np.float32)
    args = [np.asarray(a, np.float32) for a in
            (weight_xr, weight_hr, bias_r, weight_xz, weight_hz, bias_z,
             weight_x, weight_h, bias, w1, b1, w2, b2)]
    zero_bias = all(np.all(a == 0) for a in (args[2], args[5], args[8]))
    nc = _get_program(act_copy_bias=not zero_bias)
    in_maps = _pack_inputs(x, *args)
    res = run_bass_kernel_spmd(nc, in_maps, core_ids=list(range(NCORES)))
    return np.concatenate([res.results[c]["out"] for c in range(NCORES)], axis=0)



# revision 39
# speedup vs baseline: 1.0002x; 1.0002x over previous
"""GRU + MLP head kernel for Trainium2, data-parallel over batch across 8 NeuronCores.

Math (per core, batch slice BL=32):
  xr/xz/xn projections: xg[t] = W_xg @ x[t]  (PE, weight-stationary, chunked over time)
  recurrence (T=512 steps, hT layout [H->2x128 partitions, B free]):
    psum_rz = Id @ [xr_t|xz_t] ; psum_rz += W_hrz @ h       (PE)
    r = sigmoid(psum_r) ; zb = sigmoid(-psum_z) = 1-z       (ACT)
    rh = r*h                                                 (DVE)
    psum_n = Id @ xn_t ; psum_n += W_hn @ rh                 (PE)
    ht = tanh(psum_n)                                        (ACT)
    h = h + zb*(ht - h)                                      (DVE)
  head: sigmoid(h) -> w1 -> sigmoid -> w2 -> softmax
All matmul inputs bf16, PSUM accumulation fp32.
"""

import numpy as np
import ml_dtypes

BF = ml_dtypes.bfloat16
P = 128
B, T, I, H = 256, 512, 128, 256
NCORES = 8
BL = B // NCORES  # batch per core

_prog_cache = {}


def _fuse_ldweights(nc, mybir):
    """Re-fuse standalone InstLdweights into the following InstMatmult.

    Tile lowering splits every matmul into LDWEIGHTS + MATMUL; walrus only
    applies Fast Weight Load (4 XBUS, ~4x faster weight streaming) to
    self-loading matmuls, so the split leaves LDWEIGHTS at 1 col/cycle
    (~104ns per 128-col block) as the dominant PE cost in this kernel.
    Merging the pair back (ldweights=True + wait/dep transfer) lets codegen
    pick FWL. Runs pre-bacc so generate_event_semaphores re-legalizes waits.
    """
    for f in nc.m.functions:
        for bb in f.blocks:
            insts = bb.instructions
            to_remove = []
            i = 0
            n = len(insts)
            while i < n:
                ins = insts[i]
                if isinstance(ins, mybir.InstLdweights):
                    # partner matmul = next PE instruction in block order
                    j = i + 1
                    mm = None
                    while j < n:
                        nxt = insts[j]
                        if isinstance(nxt, mybir.InstMatmult):
                            mm = nxt
                            break
                        if isinstance(nxt, mybir.InstLdweights):
                            break
                        j += 1
                    if mm is not None:
                        lw = ins.sync_info
                        msi = mm.sync_info
                        waits = (list(lw.on_wait) if lw else []) + (
                            list(msi.on_wait) if msi else [])
                        upds = (list(msi.on_update) if msi else []) + (
                            list(lw.on_update) if lw else [])
                        mm.sync_info = mybir.SyncInfo(on_wait=waits, on_update=upds)
                        mm.ldweights = True
                        mm.merge_dependencies_from(ins)
                        to_remove.append(ins)
                i += 1
            for ins in to_remove:
                insts.remove(ins)


def _dedup_adjacent_ldweights(nc, mybir):
    """Remove InstLdweights that reload the exact weights AP already loaded.

    Runs on the final BIR (post nc.compile()). Walks each block's PE weight
    stream; an InstLdweights whose weights AP string-matches the previous
    load, with only InstMatmult between, is redundant — the following
    matmuls (ldweights=False post-split) use whatever is loaded. Waits on
    the removed load migrate to the next matmul when it has none (each
    instruction may carry at most one wait); pairs where both carry waits
    are left alone. Weight SBUF regions are write-once in this kernel, so
    same-AP implies same contents.
    """
    removed = 0
    for f in nc.m.functions:
        for bb in f.blocks:
            insts = bb.instructions
            last_ap = None
            last_clobber = False
            to_remove = []
            for idx, ins in enumerate(insts):
                if isinstance(ins, mybir.InstMatmult):
                    continue
                if not isinstance(ins, mybir.InstLdweights):
                    continue
                ap_s = str(ins.ins[0])
                if ap_s == last_ap:
                    si = ins.sync_info
                    waits = list(si.on_wait) if si else []
                    upds = list(si.on_update) if si else []
                    # find following matmul to absorb waits/updates
                    mm = None
                    for j in range(idx + 1, len(insts)):
                        if isinstance(insts[j], mybir.InstMatmult):
                            mm = insts[j]
                            break
                        if isinstance(insts[j], mybir.InstLdweights):
                            break
                    if mm is None:
                        last_ap = ap_s
                        continue
                    msi = mm.sync_info
                    mw = list(msi.on_wait) if msi else []
                    mu = list(msi.on_update) if msi else []
                    if waits and mw:
                        last_ap = ap_s
                        continue  # can't merge two waits; keep the load
                    mm.sync_info = mybir.SyncInfo(on_wait=waits + mw,
                                                  on_update=mu + upds)
                    mm.merge_dependencies_from(ins)
                    to_remove.append(ins)
                    removed += 1
                else:
                    last_ap = ap_s
            for ins in to_remove:
                insts.remove(ins)
    return removed


def _build_program(T_total, chunk, body, chains, act_copy_bias):
    import concourse.bacc as bacc
    import concourse.tile as tile
    import concourse.bass as bass
    from concourse import mybir

    f32 = mybir.dt.float32
    bf16 = mybir.dt.bfloat16
    AF = mybir.ActivationFunctionType
    ET = mybir.EngineType

    Bc = BL // chains
    nchunks = T_total // chunk
    nblk = chunk // 16  # projection blocks (16 timesteps each) per chunk

    nc = bacc.Bacc("TRN2", target_bir_lowering=False, debug=False)

    xT_d = nc.dram_tensor("xT", [P, BL * T_total], bf16, kind="ExternalInput")
    wrz_d = nc.dram_tensor("wrz", [P, 8 * P], bf16, kind="ExternalInput")
    wn_d = nc.dram_tensor("wn", [P, 4 * P], bf16, kind="ExternalInput")
    wx_d = nc.dram_tensor("wx", [P, 6 * P], bf16, kind="ExternalInput")
    ident_d = nc.dram_tensor("ident", [P, P], bf16, kind="ExternalInput")
    bproj_d = nc.dram_tensor("bproj", [P, 6], f32, kind="ExternalInput")
    w1T_d = nc.dram_tensor("w1T", [P, 2 * 64], bf16, kind="ExternalInput")
    w2T_d = nc.dram_tensor("w2T", [64, 10], bf16, kind="ExternalInput")
    b1_d = nc.dram_tensor("b1", [64, 1], f32, kind="ExternalInput")
    b2_d = nc.dram_tensor("b2", [1, 10], f32, kind="ExternalInput")
    out_d = nc.dram_tensor("out", [BL, 10], f32, kind="ExternalOutput")

    with tile.TileContext(nc) as tc:
        with (
            tc.tile_pool(name="consts", bufs=1) as consts,
            tc.tile_pool(name="xrzp", bufs=1) as xrzp,
            tc.tile_pool(name="state", bufs=1) as state,
            tc.tile_pool(name="work", bufs=2) as work,
            tc.tile_pool(name="ppsum", bufs=2, space="PSUM") as ppsum,
            tc.tile_pool(name="rzpsum", bufs=2 * chains, space="PSUM") as rzpsum,
            tc.tile_pool(name="npsum", bufs=2, space="PSUM") as npsum,
        ):
            # ---- load constants ----
            # xT is t-major [P, (t b)]; chunk 0 lands in per-block pieces so
            # the first projections start after ~128KB, later chunks as one
            # DMA each.
            xT_sb = consts.tile([P, BL * T_total], bf16)
            bw = 16 * BL
            for bi in range(nblk):
                nc.sync.dma_start(out=xT_sb[:, bi * bw : (bi + 1) * bw],
                                  in_=xT_d.ap()[:, bi * bw : (bi + 1) * bw])
            cw = chunk * BL
            for ci in range(1, nchunks):
                nc.sync.dma_start(out=xT_sb[:, ci * cw : (ci + 1) * cw],
                                  in_=xT_d.ap()[:, ci * cw : (ci + 1) * cw])
            wrz_sb = consts.tile([P, 8 * P], bf16)
            nc.sync.dma_start(out=wrz_sb, in_=wrz_d.ap())
            wn_sb = consts.tile([P, 4 * P], bf16)
            nc.sync.dma_start(out=wn_sb, in_=wn_d.ap())
            wx_sb = consts.tile([P, 6 * P], bf16)
            nc.sync.dma_start(out=wx_sb, in_=wx_d.ap())
            id_sb = consts.tile([P, P], bf16)
            nc.sync.dma_start(out=id_sb, in_=ident_d.ap())
            bproj_sb = consts.tile([P, 6], f32)
            nc.sync.dma_start(out=bproj_sb, in_=bproj_d.ap())
            w1T_sb = consts.tile([P, 2 * 64], bf16)
            nc.sync.dma_start(out=w1T_sb, in_=w1T_d.ap())
            w2T_sb = consts.tile([64, 10], bf16)
            nc.sync.dma_start(out=w2T_sb, in_=w2T_d.ap())
            b1_sb = consts.tile([64, 1], f32)
            nc.sync.dma_start(out=b1_sb, in_=b1_d.ap())
            b2_sb = consts.tile([1, 10], f32)
            nc.sync.dma_start(out=b2_sb, in_=b2_d.ap())
            ones_sb = consts.tile([1, BL], f32)
            nc.vector.memset(ones_sb, 1.0)

            # xT viewed [P, t, b]
            xT_v = xT_sb.rearrange("p (t b) -> p t b", b=BL)

            # xrz double buffers: [P, chunk(t), 6(gc), 32(b)] bf16
            xrz_bufs = [xrzp.tile([P, chunk, 6, BL], bf16, tag=f"xrz{i}", name=f"xrz{i}")
                        for i in range(2)]

            # hidden state ping-pong per chain
            h_tiles = [
                [state.tile([P, 2 * Bc], bf16, tag=f"h{c}_{i}", name=f"h{c}_{i}")
                 for i in range(2)]
                for c in range(chains)
            ]
            for c in range(chains):
                nc.vector.memset(h_tiles[c][0], 0.0)

            def emit_proj(cidx):
                """Projections for chunk cidx into xrz_bufs[cidx % 2]."""
                buf = xrz_bufs[cidx % 2]
                t0 = cidx * chunk
                # blk-outer so the earliest timesteps' gates finish first and
                # the recurrence can start while later blocks still project.
                for blk in range(nblk):
                    for gc in range(6):
                        g, mc = gc // 2, gc % 2
                        pp = ppsum.tile([P, 16 * BL], f32, tag="pp")
                        # rhs (t, b)-major (xT layout) -> psum cols t*BL+b:
                        # contiguous stream in, stride-friendly copy out.
                        rhs = xT_v[:, t0 + blk * 16 : t0 + (blk + 1) * 16, :]
                        nc.tensor.matmul(
                            pp, wx_sb[:, gc * P : (gc + 1) * P], rhs,
                            start=True, stop=True,
                        )
                        # copy psum [P, t, b] -> xrz[t, gc, b]; both (t, b) ordered
                        src = pp.rearrange("p (t b) -> p t b", t=16)
                        dst = buf[:, blk * 16 : (blk + 1) * 16, gc, :]
                        if act_copy_bias:
                            nc.scalar.activation(
                                dst, src, AF.Copy, bias=bproj_sb[:, gc : gc + 1]
                            )
                        elif (blk * 6 + gc) % 2 == 0:
                            nc.scalar.activation(dst, src, AF.Copy)
                        else:
                            nc.vector.tensor_copy(dst, src)

            def emit_step(tsl, xbuf, c, h_in, h_out):
                """One GRU step for chain c at dynamic time slice tsl, reading xbuf."""
                b0 = c * Bc
                # rz phase
                prz = rzpsum.tile([P, 4 * Bc], f32, tag="prz")
                rz_rhs = xbuf[:, tsl, 0:4, b0 : b0 + Bc]
                nc.tensor.matmul(prz, id_sb, rz_rhs, start=True, stop=False,
                                 skip_group_check=True)
                for mc in range(4):
                    for k in range(2):
                        nc.tensor.matmul(
                            prz[:, mc * Bc : (mc + 1) * Bc],
                            wrz_sb[:, (k * 4 + mc) * P : (k * 4 + mc + 1) * P],
                            h_in[:, k * Bc : (k + 1) * Bc],
                            start=False, stop=(k == 1),
                            skip_group_check=True,
                        )
                # z-gate weights are pre-negated host-side, so one sigmoid
                # yields [r | 1-z] together.
                rzb_sb = work.tile([P, 4 * Bc], bf16, tag=f"rzb{c}")
                nc.scalar.activation(rzb_sb, prz, AF.Sigmoid)
                r_sb = rzb_sb[:, 0 : 2 * Bc]
                zb_sb = rzb_sb[:, 2 * Bc : 4 * Bc]
                rh_sb = work.tile([P, 2 * Bc], bf16, tag=f"rh{c}")
                nc.vector.tensor_mul(rh_sb, r_sb, h_in)
                # n phase
                pn = npsum.tile([P, 2 * Bc], f32, tag="pn")
                n_rhs = xbuf[:, tsl, 4:6, b0 : b0 + Bc]
                nc.tensor.matmul(pn, id_sb, n_rhs, start=True, stop=False,
                                 skip_group_check=True)
                for mc in range(2):
                    for k in range(2):
                        nc.tensor.matmul(
                            pn[:, mc * Bc : (mc + 1) * Bc],
                            wn_sb[:, (k * 2 + mc) * P : (k * 2 + mc + 1) * P],
                            rh_sb[:, k * Bc : (k + 1) * Bc],
                            start=False, stop=(k == 1),
                            skip_group_check=True,
                        )
                ht_sb = work.tile([P, 2 * Bc], bf16, tag=f"ht{c}")
                nc.scalar.activation(ht_sb, pn, AF.Tanh)
                u_sb = work.tile([P, 2 * Bc], bf16, tag=f"u{c}")
                nc.vector.tensor_sub(u_sb, ht_sb, h_in)
                d_sb = work.tile([P, 2 * Bc], bf16, tag=f"d{c}")
                nc.vector.tensor_mul(d_sb, zb_sb, u_sb)
                nc.vector.tensor_add(h_out, h_in, d_sb)

            # ---- main schedule ----
            emit_proj(0)
            import concourse.bass as _b

            for cidx in range(nchunks):
                if cidx + 1 < nchunks:
                    emit_proj(cidx + 1)
                xbuf = xrz_bufs[cidx % 2]
                if body == 0:
                    # fully static unroll: cheap APs, no back-edge barriers
                    for s in range(chunk):
                        tsl = slice(s, s + 1)
                        for c in range(chains):
                            h_in = h_tiles[c][s % 2]
                            h_out = h_tiles[c][(s + 1) % 2]
                            emit_step(tsl, xbuf, c, h_in, h_out)
                else:
                    with tc.For_i(0, chunk, body, hint_engines=(ET.PE,)) as tv:
                        for s in range(body):
                            t_sv = tv + s if s > 0 else tv
                            tsl = _b.ds(t_sv, 1)
                            for c in range(chains):
                                h_in = h_tiles[c][s % 2]
                                h_out = h_tiles[c][(s + 1) % 2]
                                emit_step(tsl, xbuf, c, h_in, h_out)

            # ---- head ----
            for c in range(chains):
                h_fin = h_tiles[c][0]
                sh = work.tile([P, 2 * Bc], bf16, tag=f"sh{c}")
                nc.scalar.activation(sh, h_fin, AF.Sigmoid)
                p1 = npsum.tile([64, Bc], f32, tag="pn")
                for k in range(2):
                    nc.tensor.matmul(
                        p1, w1T_sb[:, k * 64 : (k + 1) * 64],
                        sh[:, k * Bc : (k + 1) * Bc],
                        start=(k == 0), stop=(k == 1), skip_group_check=True,
                    )
                s1 = work.tile([64, Bc], bf16, tag=f"s1{c}")
                nc.scalar.activation(s1, p1, AF.Sigmoid, bias=b1_sb)
                p2 = npsum.tile([Bc, 10], f32, tag="pn")
                nc.tensor.matmul(p2, ones_sb[:, c * Bc : (c + 1) * Bc], b2_sb,
                                 start=True, stop=False, skip_group_check=True)
                nc.tensor.matmul(p2, s1, w2T_sb, start=False, stop=True,
                                 skip_group_check=True)
                # softmax over free dim (10)
                lg = work.tile([Bc, 10], f32, tag=f"lg{c}")
                nc.vector.tensor_copy(lg, p2)
                mx = work.tile([Bc, 1], f32, tag=f"mx{c}")
                nc.vector.reduce_max(mx, lg, axis=mybir.AxisListType.X)
                nmx = work.tile([Bc, 1], f32, tag=f"nmx{c}")
                nc.vector.tensor_scalar_mul(nmx, mx, -1.0)
                ex = work.tile([Bc, 10], f32, tag=f"ex{c}")
                nc.scalar.activation(ex, lg, AF.Exp, bias=nmx)
                sm = work.tile([Bc, 1], f32, tag=f"sm{c}")
                nc.vector.reduce_sum(sm, ex, axis=mybir.AxisListType.X)
                ri = work.tile([Bc, 1], f32, tag=f"ri{c}")
                nc.vector.reciprocal(ri, sm)
                oo = work.tile([Bc, 10], f32, tag=f"oo{c}")
                nc.vector.tensor_scalar_mul(oo, ex, ri)
                nc.sync.dma_start(out=out_d.ap()[c * Bc : (c + 1) * Bc, :], in_=oo)

    nc.compile()
    _dedup_adjacent_ldweights(nc, mybir)
    return nc


def _get_program(T_total=T, chunk=128, body=0, chains=2, act_copy_bias=False):
    key = (T_total, chunk, body, chains, act_copy_bias)
    if key not in _prog_cache:
        _prog_cache[key] = _build_program(T_total, chunk, body, chains, act_copy_bias)
    return _prog_cache[key]


def _pack_inputs(x, weight_xr, weight_hr, bias_r, weight_xz, weight_hz, bias_z,
                 weight_x, weight_h, bias, w1, b1, w2, b2, T_total=T):
    """Host-side prep: per-core input dicts."""
    def wt_blocks(W, korder):
        # lhsT blocks [P, nk*nm... ] for W [M, K]: block (k, mc) = W[mc*128:(mc+1)*128, k*128:(k+1)*128].T
        blocks = []
        for k, mc in korder:
            blocks.append(np.ascontiguousarray(
                W[mc * P : (mc + 1) * P, k * P : (k + 1) * P].T.astype(BF)))
        return np.concatenate(blocks, axis=1)

    # wrz: order (k*4+mc): mc 0,1 -> hr rows 0/1; mc 2,3 -> -hz rows 0/1
    # (z-gate negated so sigmoid(psum_z) = 1-z directly)
    wrz_blocks = []
    for k in range(2):
        for mc in range(4):
            W = weight_hr if mc < 2 else -weight_hz
            m = mc % 2
            wrz_blocks.append(np.ascontiguousarray(
                W[m * P : (m + 1) * P, k * P : (k + 1) * P].T.astype(BF)))
    wrz = np.concatenate(wrz_blocks, axis=1)
    wn_blocks = []
    for k in range(2):
        for mc in range(2):
            wn_blocks.append(np.ascontiguousarray(
                weight_h[mc * P : (mc + 1) * P, k * P : (k + 1) * P].T.astype(BF)))
    wn = np.concatenate(wn_blocks, axis=1)
    wx_blocks = []
    for g, W in enumerate([weight_xr, -weight_xz, weight_x]):
        for mc in range(2):
            wx_blocks.append(np.ascontiguousarray(
                W[mc * P : (mc + 1) * P, :].T.astype(BF)))
    wx = np.concatenate(wx_blocks, axis=1)

    ident = np.eye(P, dtype=BF)
    bproj = np.stack([bias_r[:P], bias_r[P:], -bias_z[:P], -bias_z[P:],
                      bias[:P], bias[P:]], axis=1).astype(np.float32)
    w1T = np.concatenate([np.ascontiguousarray(w1[:, k * P : (k + 1) * P].T.astype(BF))
                          for k in range(2)], axis=1)  # [128, 2*64]
    w2T = np.ascontiguousarray(w2.T.astype(BF))  # [64, 10]
    b1c = b1.reshape(64, 1).astype(np.float32)
    b2c = b2.reshape(1, 10).astype(np.float32)

    common = dict(wrz=wrz, wn=wn, wx=wx, ident=ident, bproj=bproj,
                  w1T=w1T, w2T=w2T, b1=b1c, b2=b2c)
    in_maps = []
    for c in range(NCORES):
        xs = x[c * BL : (c + 1) * BL, :T_total, :].astype(BF)  # [BL, T, I]
        # t-major: [I, T, BL] -> [P, T*BL]
        xTc = np.ascontiguousarray(xs.transpose(2, 1, 0).reshape(P, BL * T_total))
        in_maps.append(dict(common, xT=xTc))
    return in_maps


def kernel(x, weight_xr, weight_hr, bias_r, weight_xz, weight_hz, bias_z,
           weight_x, weight_h, bias, w1, b1, w2, b2):
    from concourse.bass_utils import run_bass_kernel_spmd

    x = np.asarray(x, np.float32)
    args = [np.asarray(a, np.float32) for a in
            (weight_xr, weight_hr, bias_r, weight_xz, weight_hz, bias_z,
             weight_x, weight_h, bias, w1, b1, w2, b2)]
    zero_bias = all(np.all(a == 0) for a in (args[2], args[5], args[8]))
    nc = _get_program(act_copy_bias=not zero_bias)
    in_maps = _pack_inputs(x, *args)
    res = run_bass_kernel_spmd(nc, in_maps, core_ids=list(range(NCORES)))
    return np.concatenate([res.results[c]["out"] for c in range(NCORES)], axis=0)

